# revision 7
# baseline (speedup 1.0000x reference)
# CenterNet decode kernel for Trainium2 (Bass/Tile), 8-core data-parallel.
#
# Reference computation (per image):
#   heat = sigmoid(hm); heat *= (3x3 maxpool(heat) == heat)    # pool NMS
#   conf = max_c heat; cls = argmax_c heat
#   boxes from wh/reg + meshgrid; dets = [x1,y1,x2,y2,conf,cls] * (conf > 0.3)
#
# Device algorithm works in logit space (sigmoid is strictly monotone, so
# pooling / suppression / channel-argmax commute with it; one sigmoid at the
# end on the per-pixel winner):
#   phase 1 (channels on partitions, strip of rows in free dim):
#     pooled = separable 3x3 max (2 horizontal + 2 vertical shifted maxes)
#     d = pooled - x              (>= 0; == 0 iff local max)
#     w = x - 1e12*d              (exact x at local maxima, huge-negative else)
#   phase 2 (per 128-pixel chunk, PE-transpose to [pixel, C]):
#     conf = reduce_max_c(w)      (exact winner logit)
#     eq   = (w == conf); idx = reduce_min_c(iota - 1e6*eq)   (first-index rule)
#   assembly (pixels on partitions):
#     conf_s = sigmoid(conf); mask = conf_s > 0.3
#     dets columns from wh/reg/meshgrid, all multiplied by mask.
import os
import sys
import threading

for _p in ("/opt/trn_rl_repo", "/root/.axon_site/_ro/trn_rl_repo"):
    if os.path.isdir(_p) and _p not in sys.path:
        sys.path.insert(0, _p)

import numpy as np

from concourse import bacc, bass, masks, mybir, tile

F32 = mybir.dt.float32
I32 = mybir.dt.int32
AX = mybir.AxisListType
OP = mybir.AluOpType
ACTF = mybir.ActivationFunctionType

NEG = -1e30     # pad value (acts as -inf for maxes)
BIGM = 1e12     # suppression multiplier
BIGI = 1e6      # argmax index offset (c - BIGI exact in f32 for c < 2^19-ish)


def build_nc(Bc=4, C=80, H=256, W=256, S=16, n_devices=8):
    """Build the per-core program: inputs hm [Bc,C,H,W], wh/reg [Bc,2,H,W],
    xyv [2,H,W] (meshgrid/W consts), output dets [Bc, H*W, 6]."""
    assert H % S == 0 and (S * W) % 128 == 0 and (H * W) % 128 == 0
    n_strips = H // S
    cps = (S * W) // 128          # 128-pixel chunks per strip
    G = (H * W) // 128            # pixels per partition in assembly layout
    TPG = min(4, cps)             # chunks per PSUM tile
    assert TPG * C * 4 <= 2048 and cps % TPG == 0
    PPS = (S * W) // G            # assembly partitions covered by one strip
    assert PPS >= 1

    nc = bacc.Bacc("TRN2", target_bir_lowering=False, debug=False,
                   num_devices=n_devices)
    hm = nc.dram_tensor("hm", [Bc, C, H, W], F32, kind="ExternalInput")
    wh = nc.dram_tensor("wh", [Bc, 2, H * W], F32, kind="ExternalInput")
    reg = nc.dram_tensor("reg", [Bc, 2, H * W], F32, kind="ExternalInput")
    xyv = nc.dram_tensor("xyv", [2, H * W], F32, kind="ExternalInput")
    dets = nc.dram_tensor("dets", [Bc, H * W, 6], F32, kind="ExternalOutput")

    Wp = W + 2
    with tile.TileContext(nc) as tc:
        with (
            tc.tile_pool(name="singles", bufs=1) as singles,
            tc.tile_pool(name="xp", bufs=2) as xp_pool,
            tc.tile_pool(name="pool_tmp", bufs=3) as tmp_pool,
            tc.tile_pool(name="wv", bufs=2) as w_pool,
            tc.tile_pool(name="ph2", bufs=3) as ph2_pool,
            tc.tile_pool(name="strip_res", bufs=3) as sres_pool,
            tc.tile_pool(name="imgbuf", bufs=2) as img_pool,
            tc.tile_pool(name="asm", bufs=1) as asm_pool,
            tc.tile_pool(name="psum_t", bufs=6, space="PSUM") as psum_pool,
            tc.tile_pool(name="psum_b", bufs=2, space="PSUM") as psum_b_pool,
        ):
            ident = singles.tile([128, 128], F32)
            masks.make_identity(nc, ident[:])
            ident_c = singles.tile([C, C], F32)
            masks.make_identity(nc, ident_c[:])

            # iota over channel position, repeated TPG times: [128, TPG, C]
            iota_i = singles.tile([128, TPG * C], I32)
            nc.gpsimd.iota(iota_i[:], [[0, TPG], [1, C]], channel_multiplier=0)
            iota_f = singles.tile([128, TPG * C], F32)
            nc.vector.tensor_copy(iota_f[:], iota_i[:])

            # meshgrid constants, already divided by W/H: [128, G] each
            xvn = singles.tile([128, G], F32)
            yvn = singles.tile([128, G], F32)
            nc.sync.dma_start(xvn[:], xyv[0].rearrange("(p g) -> p g", p=128))
            nc.sync.dma_start(yvn[:], xyv[1].rearrange("(p g) -> p g", p=128))

            for b in range(Bc):
                conf_g = img_pool.tile([128, G], F32, tag="conf_g")
                idx_g = img_pool.tile([128, G], F32, tag="idx_g")

                for s in range(n_strips):
                    r0 = s * S
                    # --- load strip with 1-row halo, padded W -------------
                    xp = xp_pool.tile([C, (S + 2) * Wp], F32, tag="xp")
                    xp3 = xp[:].rearrange("c (r w) -> c r w", w=Wp)
                    # pad columns 0 and W+1 of every row
                    nc.gpsimd.memset(
                        xp3[:, :, 0:Wp:(Wp - 1)], NEG)
                    lo = max(r0 - 1, 0)
                    hi = min(r0 + S + 1, H)
                    dst_r0 = 1 - (r0 - lo)
                    nc.sync.dma_start(
                        xp3[:, dst_r0:dst_r0 + (hi - lo), 1:W + 1],
                        hm[b, :, lo:hi, :])
                    if r0 == 0:
                        nc.gpsimd.memset(xp3[:, 0, 1:W + 1], NEG)
                    if r0 + S == H:
                        nc.gpsimd.memset(xp3[:, S + 1, 1:W + 1], NEG)

                    # --- separable 3x3 max pool ---------------------------
                    m1 = tmp_pool.tile([C, (S + 2) * (W + 1)], F32, tag="pt")
                    m13 = m1[:].rearrange("c (r w) -> c r w", w=W + 1)
                    nc.vector.tensor_tensor(
                        m13[:, :, :], xp3[:, :, 0:W + 1], xp3[:, :, 1:W + 2],
                        op=OP.max)
                    hx = tmp_pool.tile([C, (S + 2) * W], F32, tag="pt")
                    hx3 = hx[:].rearrange("c (r w) -> c r w", w=W)
                    nc.vector.tensor_tensor(
                        hx3[:, :, :], m13[:, :, 0:W], m13[:, :, 1:W + 1],
                        op=OP.max)
                    mv = tmp_pool.tile([C, (S + 1) * W], F32, tag="pt")
                    mv3 = mv[:].rearrange("c (r w) -> c r w", w=W)
                    nc.vector.tensor_tensor(
                        mv3[:, :, :], hx3[:, 0:S + 1, :], hx3[:, 1:S + 2, :],
                        op=OP.max)
                    vm = tmp_pool.tile([C, S * W], F32, tag="pt")
                    vm3 = vm[:].rearrange("c (r w) -> c r w", w=W)
                    nc.vector.tensor_tensor(
                        vm3[:, :, :], mv3[:, 0:S, :], mv3[:, 1:S + 1, :],
                        op=OP.max)

                    # --- suppression: w = x - BIGM*(pooled - x) -----------
                    xr = xp3[:, 1:S + 1, 1:W + 1]
                    nc.gpsimd.tensor_tensor(vm3[:, :, :], vm3[:, :, :], xr,
                                            op=OP.subtract)
                    wv = w_pool.tile([C, S * W], F32, tag="wv")
                    wv3 = wv[:].rearrange("c (r w) -> c r w", w=W)
                    nc.vector.scalar_tensor_tensor(
                        wv3[:, :, :], vm3[:, :, :], -BIGM, xr,
                        op0=OP.mult, op1=OP.add)

                    # --- phase 2: transpose chunks, reduce over channels --
                    conf_t = sres_pool.tile([128, cps], F32, tag="conf_t")
                    idxm_t = sres_pool.tile([128, cps], F32, tag="idxm_t")
                    for g0 in range(0, cps, TPG):
                        wt = psum_pool.tile([128, TPG * C], F32, tag="wt")
                        wt3 = wt[:].rearrange("p (t c) -> p t c", c=C)
                        for t in range(TPG):
                            k = g0 + t
                            nc.tensor.transpose(
                                wt3[:, t, :].rearrange("p c -> p c"),
                                wv[:, k * 128:(k + 1) * 128],
                                ident_c[:])
                        nc.vector.tensor_reduce(
                            conf_t[:, g0:g0 + TPG], wt3[:, :, :],
                            axis=AX.X, op=OP.max)
                        eq = ph2_pool.tile([128, TPG * C], F32, tag="eq")
                        eq3 = eq[:].rearrange("p (t c) -> p t c", c=C)
                        cb = conf_t[:, g0:g0 + TPG].unsqueeze(-1) \
                            .broadcast_to((128, TPG, C))
                        nc.vector.tensor_tensor(eq3[:, :, :], wt3[:, :, :],
                                                cb, op=OP.is_equal)
                        im = ph2_pool.tile([128, TPG * C], F32, tag="im")
                        nc.vector.scalar_tensor_tensor(
                            im[:], eq[:], -BIGI, iota_f[:],
                            op0=OP.mult, op1=OP.add)
                        nc.vector.tensor_reduce(
                            idxm_t[:, g0:g0 + TPG],
                            im[:].rearrange("p (t c) -> p t c", c=C),
                            axis=AX.X, op=OP.min)

                    # --- transpose back to pixel-linear rows --------------
                    ct_ps = psum_b_pool.tile([cps, 128], F32, tag="tb")
                    nc.tensor.transpose(ct_ps[:], conf_t[:], ident[:])
                    conf_lin = sres_pool.tile([cps, 128], F32, tag="conf_lin")
                    nc.scalar.copy(conf_lin[:], ct_ps[:])
                    it_ps = psum_b_pool.tile([cps, 128], F32, tag="tb")
                    nc.tensor.transpose(it_ps[:], idxm_t[:], ident[:])
                    idx_lin = sres_pool.tile([cps, 128], F32, tag="idx_lin")
                    # undo the -BIGI offset during the PSUM->SBUF copy
                    nc.scalar.activation(idx_lin[:], it_ps[:], ACTF.Copy,
                                         bias=float(BIGI))

                    # --- scatter strip rows into per-image [128, G] -------
                    p0 = (s * S * W) // G
                    nc.sync.dma_start(
                        conf_g[p0:p0 + PPS, :] if PPS > 1 else
                        conf_g[p0:p0 + 1, :],
                        conf_lin[:].rearrange("a b -> a b"))
                    nc.sync.dma_start(
                        idx_g[p0:p0 + PPS, :] if PPS > 1 else
                        idx_g[p0:p0 + 1, :],
                        idx_lin[:].rearrange("a b -> a b"))

                # --- assembly for image b (pixels on partitions) ----------
                wh0 = asm_pool.tile([128, G], F32, tag="wh0")
                wh1 = asm_pool.tile([128, G], F32, tag="wh1")
                rg0 = asm_pool.tile([128, G], F32, tag="rg0")
                rg1 = asm_pool.tile([128, G], F32, tag="rg1")
                nc.sync.dma_start(wh0[:], wh[b, 0].rearrange("(p g) -> p g", p=128))
                nc.sync.dma_start(wh1[:], wh[b, 1].rearrange("(p g) -> p g", p=128))
                nc.sync.dma_start(rg0[:], reg[b, 0].rearrange("(p g) -> p g", p=128))
                nc.sync.dma_start(rg1[:], reg[b, 1].rearrange("(p g) -> p g", p=128))

                confs = asm_pool.tile([128, G], F32, tag="confs")
                nc.scalar.activation(confs[:], conf_g[:], ACTF.Sigmoid)
                mask = asm_pool.tile([128, G], F32, tag="mask")
                nc.vector.tensor_scalar(mask[:], confs[:], 0.3, None,
                                        op0=OP.is_gt)

                out_img = asm_pool.tile([128, G * 6], F32, tag="out_img")
                o3 = out_img[:].rearrange("p (g k) -> p g k", k=6)

                # masked center coords and half-extents
                tcx = asm_pool.tile([128, G], F32, tag="tcx")
                nc.vector.scalar_tensor_tensor(tcx[:], rg0[:], 1.0 / W, xvn[:],
                                               op0=OP.mult, op1=OP.add)
                tcy = asm_pool.tile([128, G], F32, tag="tcy")
                nc.vector.scalar_tensor_tensor(tcy[:], rg1[:], 1.0 / H, yvn[:],
                                               op0=OP.mult, op1=OP.add)
                nc.vector.tensor_tensor(tcx[:], tcx[:], mask[:], op=OP.mult)
                nc.vector.tensor_tensor(tcy[:], tcy[:], mask[:], op=OP.mult)
                hwx = asm_pool.tile([128, G], F32, tag="hwx")
                nc.vector.scalar_tensor_tensor(hwx[:], wh0[:], 0.5 / W, mask[:],
                                               op0=OP.mult, op1=OP.mult)
                hwy = asm_pool.tile([128, G], F32, tag="hwy")
                nc.vector.scalar_tensor_tensor(hwy[:], wh1[:], 0.5 / H, mask[:],
                                               op0=OP.mult, op1=OP.mult)

                nc.vector.tensor_tensor(o3[:, :, 0], tcx[:], hwx[:], op=OP.subtract)
                nc.vector.tensor_tensor(o3[:, :, 1], tcy[:], hwy[:], op=OP.subtract)
                nc.vector.tensor_tensor(o3[:, :, 2], tcx[:], hwx[:], op=OP.add)
                nc.vector.tensor_tensor(o3[:, :, 3], tcy[:], hwy[:], op=OP.add)
                nc.vector.tensor_tensor(o3[:, :, 4], confs[:], mask[:], op=OP.mult)
                nc.vector.tensor_tensor(o3[:, :, 5], idx_g[:], mask[:], op=OP.mult)

                nc.sync.dma_start(
                    dets[b].rearrange("(p g) k -> p (g k)", p=128), out_img[:])

    nc.compile()
    return nc


_CACHE = {}
_CACHE_LOCK = threading.Lock()


def _get_nc(key, **kw):
    with _CACHE_LOCK:
        if key not in _CACHE:
            _CACHE[key] = build_nc(**kw)
        return _CACHE[key]


def _xyv(H, W):
    yv, xv = np.meshgrid(np.arange(H, dtype=np.float32),
                         np.arange(W, dtype=np.float32), indexing="ij")
    return np.stack([xv / W, yv / H]).reshape(2, H * W).astype(np.float32)


def kernel(hm: np.ndarray, wh: np.ndarray, reg: np.ndarray) -> np.ndarray:
    from concourse.bass_utils import run_bass_kernel_spmd

    B, C, H, W = hm.shape
    n_cores = 8
    assert B % n_cores == 0
    Bc = B // n_cores
    nc = _get_nc(("full", Bc, C, H, W), Bc=Bc, C=C, H=H, W=W, S=16)
    xyv = _xyv(H, W)
    in_maps = []
    for i in range(n_cores):
        sl = slice(i * Bc, (i + 1) * Bc)
        in_maps.append({
            "hm": np.ascontiguousarray(hm[sl]),
            "wh": np.ascontiguousarray(wh[sl]).reshape(Bc, 2, H * W),
            "reg": np.ascontiguousarray(reg[sl]).reshape(Bc, 2, H * W),
            "xyv": xyv,
        })
    res = run_bass_kernel_spmd(nc, in_maps, core_ids=list(range(n_cores)))
    return np.concatenate([res.results[i]["dets"] for i in range(n_cores)],
                          axis=0)


# revision 18
# speedup vs baseline: 1.0213x; 1.0213x over previous
# CenterNet decode kernel for Trainium2 (Bass/Tile), 8-core data-parallel.
#
# Reference computation (per image):
#   heat = sigmoid(hm); heat *= (3x3 maxpool(heat) == heat)    # pool NMS
#   conf = max_c heat; cls = argmax_c heat
#   boxes from wh/reg + meshgrid; dets = [x1,y1,x2,y2,conf,cls] * (conf > 0.3)
#
# Device algorithm works in logit space (sigmoid is strictly monotone, so
# pooling / suppression / channel-argmax commute with it; one sigmoid at the
# end on the per-pixel winner):
#   phase 1 (channels on partitions, strip of rows in free dim):
#     pooled = separable 3x3 max (2 horizontal + 2 vertical shifted maxes)
#     d = pooled - x              (>= 0; == 0 iff local max)
#     w = x - 1e12*d              (exact x at local maxima, huge-negative else)
#   phase 2 (per 128-pixel chunk, PE-transpose to [pixel, C]):
#     conf = reduce_max_c(w)      (exact winner logit)
#     eq   = (w == conf); idx = reduce_min_c(iota - 1e6*eq)   (first-index rule)
#   assembly (pixels on partitions):
#     conf_s = sigmoid(conf); mask = conf_s > 0.3
#     dets columns from wh/reg/meshgrid, all multiplied by mask.
import os
import sys
import threading

for _p in ("/opt/trn_rl_repo", "/root/.axon_site/_ro/trn_rl_repo"):
    if os.path.isdir(_p) and _p not in sys.path:
        sys.path.insert(0, _p)

import numpy as np

from concourse import bacc, bass, masks, mybir, tile

F32 = mybir.dt.float32
I32 = mybir.dt.int32
AX = mybir.AxisListType
OP = mybir.AluOpType
ACTF = mybir.ActivationFunctionType

NEG = -1e30     # pad value (acts as -inf for maxes)
BIGM = 1e12     # suppression multiplier
BIGI = 1e6      # argmax index offset (c - BIGI exact in f32 for c < 2^19-ish)

_FLT_MAX = float(np.finfo(np.float32).max)
_CUSTOM = {}


def _custom_ops():
    """Register (once per process) the two fused DVE micro-ops:
    CN_WSEL:  w = x if x == pooled else -FLT_MAX       (suppression)
    CN_IDXC:  cand = (channel pos) if w == conf else s0  (argmax candidates,
              channel pos = Idx - SubIdx*s1 within [P, S, N] pages)"""
    if _CUSTOM:
        return _CUSTOM
    import re
    from concourse.dve_spec import (Spec, Src0, Src1, MaxNeg, select, eq,
                                    Idx, SubIdx, C0, C1)
    from concourse import dve_ops as D
    from concourse.dve_ops import DveOp, OPS

    def reg(name, spec, subdim):
        for op in OPS:
            if op.name == name:
                return op
        op = DveOp(name, spec, subdim=subdim, uops_sha={})
        OPS.append(op)
        D.CUSTOM_DVE_SPECS[name] = spec
        D._SUB_OPCODE_FOR_NAME[name] = D._CUSTOM_DVE_ROW_BASE + len(OPS) - 1
        for ver in ("v3", "v4"):
            try:
                op.compile(ver)
            except ValueError as e:
                m = re.search(r"%s: ([0-9a-f]+)" % ver, str(e))
                if m:
                    op.uops_sha[ver] = m.group(1)
                    op.compile(ver)
        return op

    _CUSTOM["wsel"] = reg(
        "CN_WSEL",
        Spec(body=select(eq(Src0, Src1), Src0, MaxNeg),
             reference=lambda in0, in1, c0=0, c1=0, c2=0: np.where(
                 in0 == in1.reshape(in0.shape), in0,
                 -_FLT_MAX).astype(in0.dtype)),
        subdim=False)
    _CUSTOM["idxc"] = reg(
        "CN_IDXC",
        Spec(body=select(eq(Src0, Src1), Idx - SubIdx * C1, C0),
             reference=lambda in0, in1, c0=0, c1=0, c2=0: np.where(
                 in0 == in1.reshape(in0.shape),
                 (np.arange(in0.shape[-2] * in0.shape[-1], dtype=np.float32)
                  .reshape(in0.shape[-2], in0.shape[-1])
                  - np.arange(in0.shape[-2], dtype=np.float32)[:, None]
                  * np.float32(np.asarray(c1).flat[0]))[None],
                 np.asarray(c0, dtype=np.float32)).astype(np.float32)),
        subdim=True)
    return _CUSTOM


def build_nc(Bc=4, C=80, H=256, W=256, S=16, n_devices=8, reps=1):
    """Build the per-core program: inputs hm [Bc,C,H,W], wh/reg [Bc,2,H,W],
    xyv [2,H,W] (meshgrid/W consts), output dets [Bc, H*W, 6]."""
    assert H % S == 0 and (S * W) % 128 == 0 and (H * W) % 128 == 0
    n_strips = H // S
    cps = (S * W) // 128          # 128-pixel chunks per strip
    G = (H * W) // 128            # pixels per partition in assembly layout
    TPG = min(4, cps)             # chunks per PSUM tile
    assert TPG * C * 4 <= 2048 and cps % TPG == 0
    PPS = (S * W) // G            # assembly partitions covered by one strip
    assert PPS >= 1

    cust = _custom_ops()
    nc = bacc.Bacc("TRN2", target_bir_lowering=False, debug=False,
                   num_devices=n_devices)
    hm = nc.dram_tensor("hm", [Bc, C, H, W], F32, kind="ExternalInput")
    wh = nc.dram_tensor("wh", [Bc, 2, H * W], F32, kind="ExternalInput")
    reg = nc.dram_tensor("reg", [Bc, 2, H * W], F32, kind="ExternalInput")
    xyv = nc.dram_tensor("xyv", [2, H * W], F32, kind="ExternalInput")
    dets = nc.dram_tensor("dets", [Bc, H * W, 6], F32, kind="ExternalOutput")

    Wp = W + 2
    with tile.TileContext(nc) as tc:
        with (
            tc.tile_pool(name="singles", bufs=1) as singles,
            tc.tile_pool(name="xp", bufs=2) as xp_pool,
            tc.tile_pool(name="pool_tmp", bufs=3) as tmp_pool,
            tc.tile_pool(name="wv", bufs=2) as w_pool,
            tc.tile_pool(name="ph2", bufs=3) as ph2_pool,
            tc.tile_pool(name="strip_res", bufs=3) as sres_pool,
            tc.tile_pool(name="imgbuf", bufs=2) as img_pool,
            tc.tile_pool(name="asm", bufs=1) as asm_pool,
            tc.tile_pool(name="psum_t", bufs=6, space="PSUM") as psum_pool,
            tc.tile_pool(name="psum_b", bufs=2, space="PSUM") as psum_b_pool,
        ):
            ident = singles.tile([128, 128], F32)
            masks.make_identity(nc, ident[:])
            ident_c = singles.tile([C, C], F32)
            masks.make_identity(nc, ident_c[:])

            # meshgrid constants, already divided by W/H: [128, G] each
            xvn = singles.tile([128, G], F32)
            yvn = singles.tile([128, G], F32)
            nc.sync.dma_start(xvn[:], xyv[0].rearrange("(p g) -> p g", p=128))
            nc.sync.dma_start(yvn[:], xyv[1].rearrange("(p g) -> p g", p=128))

            for _rep in range(reps):
              for b in range(Bc):
                conf_g = img_pool.tile([128, G], F32, tag="conf_g")
                idx_g = img_pool.tile([128, G], F32, tag="idx_g")

                for s in range(n_strips):
                    r0 = s * S
                    # --- load strip with 1-row halo, padded W -------------
                    xp = xp_pool.tile([C, (S + 2) * Wp], F32, tag="xp")
                    xp3 = xp[:].rearrange("c (r w) -> c r w", w=Wp)
                    # pad columns 0 and W+1 of every row
                    nc.gpsimd.memset(
                        xp3[:, :, 0:Wp:(Wp - 1)], NEG)
                    lo = max(r0 - 1, 0)
                    hi = min(r0 + S + 1, H)
                    dst_r0 = 1 - (r0 - lo)
                    nc.sync.dma_start(
                        xp3[:, dst_r0:dst_r0 + (hi - lo), 1:W + 1],
                        hm[b, :, lo:hi, :])
                    if r0 == 0:
                        nc.gpsimd.memset(xp3[:, 0, 1:W + 1], NEG)
                    if r0 + S == H:
                        nc.gpsimd.memset(xp3[:, S + 1, 1:W + 1], NEG)

                    # --- separable 3x3 max pool ---------------------------
                    m1 = tmp_pool.tile([C, (S + 2) * (W + 1)], F32, tag="pt")
                    m13 = m1[:].rearrange("c (r w) -> c r w", w=W + 1)
                    nc.vector.tensor_tensor(
                        m13[:, :, :], xp3[:, :, 0:W + 1], xp3[:, :, 1:W + 2],
                        op=OP.max)
                    hx = tmp_pool.tile([C, (S + 2) * W], F32, tag="pt")
                    hx3 = hx[:].rearrange("c (r w) -> c r w", w=W)
                    nc.vector.tensor_tensor(
                        hx3[:, :, :], m13[:, :, 0:W], m13[:, :, 1:W + 1],
                        op=OP.max)
                    mv = tmp_pool.tile([C, (S + 1) * W], F32, tag="pt")
                    mv3 = mv[:].rearrange("c (r w) -> c r w", w=W)
                    nc.vector.tensor_tensor(
                        mv3[:, :, :], hx3[:, 0:S + 1, :], hx3[:, 1:S + 2, :],
                        op=OP.max)
                    vm = tmp_pool.tile([C, S * W], F32, tag="pt")
                    vm3 = vm[:].rearrange("c (r w) -> c r w", w=W)
                    nc.vector.tensor_tensor(
                        vm3[:, :, :], mv3[:, 0:S, :], mv3[:, 1:S + 1, :],
                        op=OP.max)

                    # --- suppression: w = x if x == pooled else -FLT_MAX --
                    xr = xp3[:, 1:S + 1, 1:W + 1]
                    wv = w_pool.tile([C, S * W], F32, tag="wv")
                    wv3 = wv[:].rearrange("c (r w) -> c r w", w=W)
                    nc.vector._custom_dve(cust["wsel"], out=wv3[:, :, :],
                                          in0=xr, in1=vm3[:, :, :])

                    # --- phase 2: transpose chunks, reduce over channels --
                    conf_t = sres_pool.tile([128, cps], F32, tag="conf_t")
                    idxm_t = sres_pool.tile([128, cps], F32, tag="idxm_t")
                    for g0 in range(0, cps, TPG):
                        wt = psum_pool.tile([128, TPG * C], F32, tag="wt")
                        wt3 = wt[:].rearrange("p (t c) -> p t c", c=C)
                        for t in range(TPG):
                            k = g0 + t
                            nc.tensor.transpose(
                                wt3[:, t, :].rearrange("p c -> p c"),
                                wv[:, k * 128:(k + 1) * 128],
                                ident_c[:])
                        nc.vector.tensor_reduce(
                            conf_t[:, g0:g0 + TPG], wt3[:, :, :],
                            axis=AX.X, op=OP.max)
                        cb = conf_t[:, g0:g0 + TPG].unsqueeze(-1) \
                            .broadcast_to((128, TPG, C))
                        im = ph2_pool.tile([128, TPG * C], F32, tag="im")
                        im3 = im[:].rearrange("p (t c) -> p t c", c=C)
                        nc.vector._custom_dve(cust["idxc"], out=im3[:, :, :],
                                              in0=wt3[:, :, :], in1=cb,
                                              s0=1e4, s1=float(C))
                        nc.vector.tensor_reduce(
                            idxm_t[:, g0:g0 + TPG], im3[:, :, :],
                            axis=AX.X, op=OP.min)

                    # --- transpose back to pixel-linear rows --------------
                    ct_ps = psum_b_pool.tile([cps, 128], F32, tag="tb")
                    nc.tensor.transpose(ct_ps[:], conf_t[:], ident[:])
                    conf_lin = sres_pool.tile([cps, 128], F32, tag="conf_lin")
                    nc.scalar.copy(conf_lin[:], ct_ps[:])
                    it_ps = psum_b_pool.tile([cps, 128], F32, tag="tb")
                    nc.tensor.transpose(it_ps[:], idxm_t[:], ident[:])
                    idx_lin = sres_pool.tile([cps, 128], F32, tag="idx_lin")
                    nc.scalar.copy(idx_lin[:], it_ps[:])

                    # --- scatter strip rows into per-image [128, G] -------
                    p0 = (s * S * W) // G
                    nc.sync.dma_start(
                        conf_g[p0:p0 + PPS, :] if PPS > 1 else
                        conf_g[p0:p0 + 1, :],
                        conf_lin[:].rearrange("a b -> a b"))
                    nc.sync.dma_start(
                        idx_g[p0:p0 + PPS, :] if PPS > 1 else
                        idx_g[p0:p0 + 1, :],
                        idx_lin[:].rearrange("a b -> a b"))

                # --- assembly for image b (pixels on partitions) ----------
                wh0 = asm_pool.tile([128, G], F32, tag="wh0")
                wh1 = asm_pool.tile([128, G], F32, tag="wh1")
                rg0 = asm_pool.tile([128, G], F32, tag="rg0")
                rg1 = asm_pool.tile([128, G], F32, tag="rg1")
                nc.sync.dma_start(wh0[:], wh[b, 0].rearrange("(p g) -> p g", p=128))
                nc.sync.dma_start(wh1[:], wh[b, 1].rearrange("(p g) -> p g", p=128))
                nc.sync.dma_start(rg0[:], reg[b, 0].rearrange("(p g) -> p g", p=128))
                nc.sync.dma_start(rg1[:], reg[b, 1].rearrange("(p g) -> p g", p=128))

                confs = asm_pool.tile([128, G], F32, tag="confs")
                nc.scalar.activation(confs[:], conf_g[:], ACTF.Sigmoid)
                mask = asm_pool.tile([128, G], F32, tag="mask")
                nc.vector.tensor_scalar(mask[:], confs[:], 0.3, None,
                                        op0=OP.is_gt)

                out_img = asm_pool.tile([128, G * 6], F32, tag="out_img")
                o3 = out_img[:].rearrange("p (g k) -> p g k", k=6)

                # masked center coords and half-extents
                tcx = asm_pool.tile([128, G], F32, tag="tcx")
                nc.vector.scalar_tensor_tensor(tcx[:], rg0[:], 1.0 / W, xvn[:],
                                               op0=OP.mult, op1=OP.add)
                tcy = asm_pool.tile([128, G], F32, tag="tcy")
                nc.vector.scalar_tensor_tensor(tcy[:], rg1[:], 1.0 / H, yvn[:],
                                               op0=OP.mult, op1=OP.add)
                nc.vector.tensor_tensor(tcx[:], tcx[:], mask[:], op=OP.mult)
                nc.vector.tensor_tensor(tcy[:], tcy[:], mask[:], op=OP.mult)
                hwx = asm_pool.tile([128, G], F32, tag="hwx")
                nc.vector.scalar_tensor_tensor(hwx[:], wh0[:], 0.5 / W, mask[:],
                                               op0=OP.mult, op1=OP.mult)
                hwy = asm_pool.tile([128, G], F32, tag="hwy")
                nc.vector.scalar_tensor_tensor(hwy[:], wh1[:], 0.5 / H, mask[:],
                                               op0=OP.mult, op1=OP.mult)

                nc.vector.tensor_tensor(o3[:, :, 0], tcx[:], hwx[:], op=OP.subtract)
                nc.vector.tensor_tensor(o3[:, :, 1], tcy[:], hwy[:], op=OP.subtract)
                nc.vector.tensor_tensor(o3[:, :, 2], tcx[:], hwx[:], op=OP.add)
                nc.vector.tensor_tensor(o3[:, :, 3], tcy[:], hwy[:], op=OP.add)
                nc.vector.tensor_tensor(o3[:, :, 4], confs[:], mask[:], op=OP.mult)
                nc.vector.tensor_tensor(o3[:, :, 5], idx_g[:], mask[:], op=OP.mult)

                nc.sync.dma_start(
                    dets[b].rearrange("(p g) k -> p (g k)", p=128), out_img[:])

    nc.compile()
    return nc


_CACHE = {}
_CACHE_LOCK = threading.Lock()


def _get_nc(key, **kw):
    with _CACHE_LOCK:
        if key not in _CACHE:
            _CACHE[key] = build_nc(**kw)
        return _CACHE[key]


def _xyv(H, W):
    yv, xv = np.meshgrid(np.arange(H, dtype=np.float32),
                         np.arange(W, dtype=np.float32), indexing="ij")
    return np.stack([xv / W, yv / H]).reshape(2, H * W).astype(np.float32)


def kernel(hm: np.ndarray, wh: np.ndarray, reg: np.ndarray) -> np.ndarray:
    from concourse.bass_utils import run_bass_kernel_spmd

    B, C, H, W = hm.shape
    n_cores = 8
    assert B % n_cores == 0
    Bc = B // n_cores
    nc = _get_nc(("full", Bc, C, H, W), Bc=Bc, C=C, H=H, W=W, S=16)
    xyv = _xyv(H, W)
    in_maps = []
    for i in range(n_cores):
        sl = slice(i * Bc, (i + 1) * Bc)
        in_maps.append({
            "hm": np.ascontiguousarray(hm[sl]),
            "wh": np.ascontiguousarray(wh[sl]).reshape(Bc, 2, H * W),
            "reg": np.ascontiguousarray(reg[sl]).reshape(Bc, 2, H * W),
            "xyv": xyv,
        })
    res = run_bass_kernel_spmd(nc, in_maps, core_ids=list(range(n_cores)))
    return np.concatenate([res.results[i]["dets"] for i in range(n_cores)],
                          axis=0)
